# revision 31
# baseline (speedup 1.0000x reference)
"""Trainium2 Bass kernel for nn_Attention (LayerNorm + MHA + out-proj).

Sharding: 8 cores = 4 batch elements x 2 query-halves. Each core receives its
batch element's full token sequence (rolled so its 1024 query rows are first),
computes LayerNorm + K/V projections over all 2048 tokens, Q projection and
attention for its 1024 query rows, and the output projection. No collectives.

Layout strategy (single SPMD Bass program, feature-on-partition style):
  x [2048,512] --LN--> xc(bf16) --PE transpose--> xnT [4][128d, 2048tok] bf16
  Kt/Qt per head-pair: [128(2x64), tok] bf16  (W chunk stationary, xnT moving)
  V:                   [128tok, 8x65] bf16    (xnT stationary, W moving;
                                               65th col = ones for row-sums)
  St per (pair,qt,kc): [128k, 2x512q] PSUM    (Kt stationary, Qt moving)
  A = exp(St/8)        ACT -> bf16 SBUF
  O^T per head:        [65, 512q] PSUM accum  (V stationary, A moving)
  normalize: row 64 -> reciprocal_approx_fast -> gpsimd bcast -> DVE mult
  out^T = W_out^T @ O^T (bf16) + b_out; host transposes back.

Scheduling: K/Q/V projections are interleaved with the LayerNorm token groups
(PE fills the otherwise DVE-bound LN window), and the output projection for
each q-tile is interleaved with the next q-tile's attention.
"""

import numpy as np
import ml_dtypes

import concourse.bass as bass
import concourse.tile as tile
from concourse import bacc, mybir
from concourse.bass_utils import run_bass_kernel_spmd
from concourse.masks import make_identity

F32 = mybir.dt.float32
BF16 = mybir.dt.bfloat16
ADD = mybir.AluOpType.add

B, N, D = 4, 2048, 512
H, DH = 8, 64
NQ = N // 2          # query rows per core
SCALE = DH ** -0.5   # 0.125
NCORES = 8

QT = NQ // 512       # 2 query tiles of 512
KC = N // 128        # 16 key chunks of 128
TT = N // 128        # 16 token tiles of 128
DC = D // 128        # 4 feature chunks of 128


def build_program(out_dtype=F32):
    nc = bacc.Bacc("TRN2", target_bir_lowering=False, debug=False)

    x_ap = nc.dram_tensor("x", [N, D], F32, kind="ExternalInput").ap()
    wqkv_ap = nc.dram_tensor("w_qkv", [D, 3 * D], BF16, kind="ExternalInput").ap()
    wout_ap = nc.dram_tensor("w_out", [D, D], BF16, kind="ExternalInput").ap()
    bout_ap = nc.dram_tensor("b_out", [D], F32, kind="ExternalInput").ap()
    gamma_ap = nc.dram_tensor("gamma", [D], F32, kind="ExternalInput").ap()
    beta_ap = nc.dram_tensor("beta", [D], F32, kind="ExternalInput").ap()
    y_ap = nc.dram_tensor("y_t", [D, NQ], out_dtype, kind="ExternalOutput").ap()

    with tile.TileContext(nc) as tc:
        attention_kernel(tc, y_ap, x_ap, wqkv_ap, wout_ap, bout_ap, gamma_ap, beta_ap)
    nc.compile()
    return nc


def attention_kernel(tc, y_ap, x_ap, wqkv_ap, wout_ap, bout_ap, gamma_ap, beta_ap):
    nc = tc.nc
    from contextlib import ExitStack

    with ExitStack() as ctx:
        persist = ctx.enter_context(tc.tile_pool(name="persist", bufs=1))
        work = ctx.enter_context(tc.tile_pool(name="work", bufs=3))

        # ---- input tiles first: get x DMAs onto the queues before weights ----
        xts = []
        for tt in range(TT):
            xt = work.tile([128, D], F32, tag="xt", bufs=16, name="xt")
            nc.sync.dma_start(xt, x_ap[tt * 128:(tt + 1) * 128, :])
            xts.append(xt)

        # ---- constants ----
        identity = persist.tile([128, 128], BF16)
        make_identity(nc, identity)
        eps_sb = persist.tile([128, 1], F32)
        nc.vector.memset(eps_sb, 1e-5)
        gamma_sb = persist.tile([128, DC], F32)
        nc.sync.dma_start(gamma_sb, gamma_ap.rearrange("(c p) -> p c", p=128))
        beta_sb = persist.tile([128, DC], F32)
        nc.sync.dma_start(beta_sb, beta_ap.rearrange("(c p) -> p c", p=128))
        bias_sb = persist.tile([128, DC], F32)
        nc.sync.dma_start(bias_sb, bout_ap.rearrange("(c p) -> p c", p=128))
        wq_sb = persist.tile([128, DC, 3 * D], BF16)
        for dc in range(DC):
            nc.sync.dma_start(wq_sb[:, dc, :], wqkv_ap[dc * 128:(dc + 1) * 128, :])
        wo_sb = persist.tile([128, DC, D], BF16)
        for p in range(4):
            nc.sync.dma_start(wo_sb[:, p, :], wout_ap[p * 128:(p + 1) * 128, :])

        # ---- persistent activations ----
        xnT = persist.tile([128, DC, N], BF16)        # [d-part, dchunk, tok]
        kt_all = persist.tile([128, 4, N], BF16)      # [2x64 head rows, pair, tok]
        qt_all = persist.tile([128, 4, NQ], BF16)     # [2x64 head rows, pair, qtok]
        v_all = persist.tile([128, TT, H * (DH + 1)], BF16)  # [tok, tt, 8x65]
        ot_all = persist.tile([128, 4, NQ], BF16)     # [2x64 inner rows, pair, qtok]

        nc.gpsimd.memset(v_all, 1.0)

        # ---- Phase A+B: LayerNorm groups interleaved with projections ----
        with (
            tc.tile_pool(name="tp_psum", bufs=4, space="PSUM") as tp_pool,
            tc.tile_pool(name="pj_psum", bufs=3, space="PSUM") as pj_pool,
        ):
            for g4 in range(TT // 4):
                # LayerNorm for 4 token tiles (DVE-heavy)
                xcs = []
                for i in range(4):
                    tt = 4 * g4 + i
                    xt = xts[tt]
                    stats = work.tile([128, 6], F32, tag="stats", bufs=8, name="stats")
                    nc.vector.bn_stats(out=stats, in_=xt)
                    mv = work.tile([128, 2], F32, tag="mv", bufs=8, name="mv")
                    nc.vector.bn_aggr(out=mv, in_=stats)
                    rstd = work.tile([128, 1], F32, tag="rstd", bufs=8, name="rstd")
                    nc.scalar.activation(
                        out=rstd, in_=mv[:, 1:2],
                        func=mybir.ActivationFunctionType.Sqrt,
                        bias=eps_sb, scale=1.0,
                    )
                    nc.vector.reciprocal(out=rstd, in_=rstd)
                    xc = work.tile([128, D], BF16, tag="xc", bufs=8, name="xc")
                    nc.vector.tensor_scalar(
                        out=xc, in0=xt, scalar1=mv[:, 0:1], scalar2=rstd,
                        op0=mybir.AluOpType.subtract, op1=mybir.AluOpType.mult,
                    )
                    xcs.append(xc)
                for dc in range(DC):
                    tpg = tp_pool.tile([128, 512], BF16, name="tpg")
                    for i in range(4):
                        nc.tensor.transpose(
                            tpg[:, i * 128:(i + 1) * 128],
                            xcs[i][:, dc * 128:(dc + 1) * 128], identity,
                        )
                    nc.scalar.activation(
                        out=xnT[:, dc, g4 * 512:(g4 + 1) * 512], in_=tpg,
                        func=mybir.ActivationFunctionType.Identity,
                        bias=beta_sb[:, dc:dc + 1], scale=gamma_sb[:, dc:dc + 1],
                    )
                # projections for this 512-token range (PE-heavy)
                t4 = g4
                for p in range(4):  # K for head pair p
                    pk = pj_pool.tile([128, 512], F32, tag="proj", name="pk")
                    for dc in range(DC):
                        nc.tensor.matmul(
                            pk,
                            lhsT=wq_sb[:, dc, D + p * 128:D + (p + 1) * 128],
                            rhs=xnT[:, dc, t4 * 512:(t4 + 1) * 512],
                            start=(dc == 0), stop=(dc == DC - 1),
                        )
                    nc.scalar.copy(kt_all[:, p, t4 * 512:(t4 + 1) * 512], pk)
                if t4 < NQ // 512:
                    for p in range(4):  # Q for head pair p
                        pq = pj_pool.tile([128, 512], F32, tag="proj", name="pq")
                        for dc in range(DC):
                            nc.tensor.matmul(
                                pq,
                                lhsT=wq_sb[:, dc, p * 128:(p + 1) * 128],
                                rhs=xnT[:, dc, t4 * 512:(t4 + 1) * 512],
                                start=(dc == 0), stop=(dc == DC - 1),
                            )
                        nc.scalar.copy(qt_all[:, p, t4 * 512:(t4 + 1) * 512], pq)
                for tt in range(4 * g4, 4 * g4 + 4):  # V for all 8 heads
                    pv = pj_pool.tile([128, 512], F32, tag="proj", name="pv")
                    for dc in range(DC):
                        nc.tensor.matmul(
                            pv,
                            lhsT=xnT[:, dc, tt * 128:(tt + 1) * 128],
                            rhs=wq_sb[:, dc, 2 * D:3 * D],
                            start=(dc == 0), stop=(dc == DC - 1),
                        )
                    nc.vector.tensor_copy(
                        v_all[:, tt, :].rearrange("p (h e) -> p h e", e=DH + 1)[:, :, 0:DH],
                        pv.rearrange("p (h d) -> p h d", d=DH),
                    )

        # ---- Phase C: attention (out-proj interleaved on the spare bank) ----
        with (
            tc.tile_pool(name="st_psum", bufs=2, space="PSUM") as st_pool,
            tc.tile_pool(name="o_psum", bufs=3, space="PSUM") as o_pool,
            tc.tile_pool(name="y_psum", bufs=1, space="PSUM") as y_pool,
        ):
            for qt in range(QT):
                for p in range(4):
                    oacc = [
                        o_pool.tile([DH + 1, 512], F32, tag="o", name=f"o{i}")
                        for i in range(2)
                    ]
                    prev_at = None
                    for kc in range(KC + 1):
                        if kc < KC:
                            st = st_pool.tile([128, 1024], F32, name="st")
                            for half in range(2):
                                nc.tensor.matmul(
                                    st[:, half * 512:(half + 1) * 512],
                                    lhsT=kt_all[64 * half:64 * half + 64, p,
                                                kc * 128:(kc + 1) * 128],
                                    rhs=qt_all[64 * half:64 * half + 64, p,
                                               qt * 512:(qt + 1) * 512],
                                    start=True, stop=True,
                                )
                        if prev_at is not None:
                            pkc = kc - 1
                            for half in range(2):
                                h = 2 * p + half
                                nc.tensor.matmul(
                                    oacc[half],
                                    lhsT=v_all[:, pkc, h * (DH + 1):(h + 1) * (DH + 1)],
                                    rhs=prev_at[:, half * 512:(half + 1) * 512],
                                    start=(pkc == 0), stop=(pkc == KC - 1),
                                )
                        if kc < KC:
                            at = work.tile([128, 1024], BF16, tag="at", bufs=4, name="at")
                            nc.scalar.activation(
                                out=at, in_=st,
                                func=mybir.ActivationFunctionType.Exp, scale=SCALE,
                            )
                            prev_at = at
                    for half in range(2):
                        o_acc = oacc[half]
                        s_sb = work.tile([1, 512], F32, tag="s_sb", bufs=3, name="s_sb")
                        nc.vector.tensor_copy(s_sb, o_acc[DH:DH + 1, :])
                        r_sb = work.tile([1, 512], F32, tag="r_sb", bufs=3, name="r_sb")
                        nc.vector.reciprocal_approx_fast(out=r_sb, in_=s_sb)
                        cb_sb = work.tile([DH, 512], F32, tag="cb", bufs=3, name="cb")
                        nc.gpsimd.partition_broadcast(cb_sb, r_sb)
                        nc.vector.tensor_tensor(
                            ot_all[64 * half:64 * half + 64, p,
                                   qt * 512:(qt + 1) * 512],
                            o_acc[0:DH, :], cb_sb, mybir.AluOpType.mult,
                        )
                # output projection for this q-tile
                for dm in range(DC):
                    yp = y_pool.tile([128, 512], F32, name="yp")
                    for p in range(4):
                        nc.tensor.matmul(
                            yp,
                            lhsT=wo_sb[:, p, dm * 128:(dm + 1) * 128],
                            rhs=ot_all[:, p, qt * 512:(qt + 1) * 512],
                            start=(p == 0), stop=(p == 3),
                        )
                    y_sb = work.tile([128, 512], F32, tag="y_sb", bufs=3, name="y_sb")
                    nc.vector.tensor_scalar_add(y_sb, yp, bias_sb[:, dm:dm + 1])
                    nc.sync.dma_start(
                        y_ap[dm * 128:(dm + 1) * 128, qt * 512:(qt + 1) * 512], y_sb
                    )


_CACHED_NC = None


def _get_program():
    global _CACHED_NC
    if _CACHED_NC is None:
        _CACHED_NC = build_program()
    return _CACHED_NC


def make_in_maps(x, ln_gamma, ln_beta, W_qkv, W_out, b_out):
    x = np.asarray(x, dtype=np.float32)
    wqkv_bf = np.asarray(W_qkv, dtype=np.float32).astype(ml_dtypes.bfloat16)
    wout_bf = np.asarray(W_out, dtype=np.float32).astype(ml_dtypes.bfloat16)
    bout = np.asarray(b_out, dtype=np.float32)
    gamma = np.asarray(ln_gamma, dtype=np.float32)
    beta = np.asarray(ln_beta, dtype=np.float32)
    in_maps = []
    for c in range(NCORES):
        b, qh = c // 2, c % 2
        xb = np.roll(x[b], -NQ * qh, axis=0)  # query rows first
        in_maps.append({
            "x": np.ascontiguousarray(xb),
            "w_qkv": wqkv_bf,
            "w_out": wout_bf,
            "b_out": bout,
            "gamma": gamma,
            "beta": beta,
        })
    return in_maps


def kernel(x, ln_gamma, ln_beta, W_qkv, W_out, b_out):
    nc = _get_program()
    in_maps = make_in_maps(x, ln_gamma, ln_beta, W_qkv, W_out, b_out)
    res = run_bass_kernel_spmd(nc, in_maps, core_ids=list(range(NCORES)))

    y = np.empty((B, N, D), dtype=np.float32)
    for c in range(NCORES):
        b, qh = c // 2, c % 2
        y[b, NQ * qh:NQ * (qh + 1), :] = res.results[c]["y_t"].T
    return y


# revision 32
# speedup vs baseline: 1.0223x; 1.0223x over previous
"""Trainium2 Bass kernel for nn_Attention (LayerNorm + MHA + out-proj).

Sharding: 8 cores = 4 batch elements x 2 query-halves. Each core receives its
batch element's full token sequence (rolled so its 1024 query rows are first),
computes LayerNorm + K/V projections over all 2048 tokens, Q projection and
attention for its 1024 query rows, and the output projection. No collectives.

Layout strategy (single SPMD Bass program, feature-on-partition style):
  x [2048,512] --LN--> xc(bf16) --PE transpose--> xnT [4][128d, 2048tok] bf16
  Kt/Qt per head-pair: [128(2x64), tok] bf16  (W chunk stationary, xnT moving)
  V:                   [128tok, 8x65] bf16    (xnT stationary, W moving;
                                               65th col = ones for row-sums)
  St per (pair,qt,kc): [128k, 2x512q] PSUM    (Kt stationary, Qt moving)
  A = exp(St/8)        ACT -> bf16 SBUF
  O^T per head:        [65, 512q] PSUM accum  (V stationary, A moving)
  normalize: row 64 -> reciprocal_approx_fast -> gpsimd bcast -> DVE mult
  out^T = W_out^T @ O^T (bf16) + b_out; host transposes back.

Scheduling: K/Q/V projections are interleaved with the LayerNorm token groups
(PE fills the otherwise DVE-bound LN window), and the output projection for
each q-tile is interleaved with the next q-tile's attention.
"""

import numpy as np
import ml_dtypes

import concourse.bass as bass
import concourse.tile as tile
from concourse import bacc, mybir
from concourse.bass_utils import run_bass_kernel_spmd
from concourse.masks import make_identity

F32 = mybir.dt.float32
BF16 = mybir.dt.bfloat16
ADD = mybir.AluOpType.add

B, N, D = 4, 2048, 512
H, DH = 8, 64
NQ = N // 2          # query rows per core
SCALE = DH ** -0.5   # 0.125
NCORES = 8

QT = NQ // 512       # 2 query tiles of 512
KC = N // 128        # 16 key chunks of 128
TT = N // 128        # 16 token tiles of 128
DC = D // 128        # 4 feature chunks of 128


def build_program(out_dtype=F32):
    nc = bacc.Bacc("TRN2", target_bir_lowering=False, debug=False)

    x_ap = nc.dram_tensor("x", [N, D], F32, kind="ExternalInput").ap()
    wqkv_ap = nc.dram_tensor("w_qkv", [D, 3 * D], BF16, kind="ExternalInput").ap()
    wout_ap = nc.dram_tensor("w_out", [D, D], BF16, kind="ExternalInput").ap()
    bout_ap = nc.dram_tensor("b_out", [D], F32, kind="ExternalInput").ap()
    gamma_ap = nc.dram_tensor("gamma", [D], F32, kind="ExternalInput").ap()
    beta_ap = nc.dram_tensor("beta", [D], F32, kind="ExternalInput").ap()
    y_ap = nc.dram_tensor("y_t", [D, NQ], out_dtype, kind="ExternalOutput").ap()

    with tile.TileContext(nc) as tc:
        attention_kernel(tc, y_ap, x_ap, wqkv_ap, wout_ap, bout_ap, gamma_ap, beta_ap)
    nc.compile()
    return nc


def attention_kernel(tc, y_ap, x_ap, wqkv_ap, wout_ap, bout_ap, gamma_ap, beta_ap):
    nc = tc.nc
    from contextlib import ExitStack

    with ExitStack() as ctx:
        persist = ctx.enter_context(tc.tile_pool(name="persist", bufs=1))
        work = ctx.enter_context(tc.tile_pool(name="work", bufs=3))

        # ---- input tiles first: get x DMAs onto the queues before weights ----
        xts = []
        for tt in range(TT):
            xt = work.tile([128, D], F32, tag="xt", bufs=16, name="xt")
            nc.sync.dma_start(xt, x_ap[tt * 128:(tt + 1) * 128, :])
            xts.append(xt)

        # ---- constants ----
        identity = persist.tile([128, 128], BF16)
        make_identity(nc, identity)
        eps_sb = persist.tile([128, 1], F32)
        nc.vector.memset(eps_sb, 1e-5)
        gamma_sb = persist.tile([128, DC], F32)
        nc.sync.dma_start(gamma_sb, gamma_ap.rearrange("(c p) -> p c", p=128))
        beta_sb = persist.tile([128, DC], F32)
        nc.sync.dma_start(beta_sb, beta_ap.rearrange("(c p) -> p c", p=128))
        bias_sb = persist.tile([128, DC], F32)
        nc.sync.dma_start(bias_sb, bout_ap.rearrange("(c p) -> p c", p=128))
        wq_sb = persist.tile([128, DC, 3 * D], BF16)
        for dc in range(DC):
            nc.sync.dma_start(wq_sb[:, dc, :], wqkv_ap[dc * 128:(dc + 1) * 128, :])
        wo_sb = persist.tile([128, DC, D], BF16)
        for p in range(4):
            nc.sync.dma_start(wo_sb[:, p, :], wout_ap[p * 128:(p + 1) * 128, :])

        # ---- persistent activations ----
        xnT = persist.tile([128, DC, N], BF16)        # [d-part, dchunk, tok]
        kt_all = persist.tile([128, 4, N], BF16)      # [2x64 head rows, pair, tok]
        qt_all = persist.tile([128, 4, NQ], BF16)     # [2x64 head rows, pair, qtok]
        v_all = persist.tile([128, TT, H * (DH + 1)], BF16)  # [tok, tt, 8x65]
        ot_all = persist.tile([128, 4, NQ], BF16)     # [2x64 inner rows, pair, qtok]

        nc.gpsimd.memset(v_all, 1.0)

        # ---- Phase A+B: LayerNorm groups interleaved with projections ----
        with (
            tc.tile_pool(name="tp_psum", bufs=4, space="PSUM") as tp_pool,
            tc.tile_pool(name="pj_psum", bufs=3, space="PSUM") as pj_pool,
        ):
            for g4 in range(TT // 4):
                # LayerNorm for 4 token tiles (DVE-heavy)
                xcs = []
                for i in range(4):
                    tt = 4 * g4 + i
                    xt = xts[tt]
                    stats = work.tile([128, 6], F32, tag="stats", bufs=8, name="stats")
                    nc.vector.bn_stats(out=stats, in_=xt)
                    mv = work.tile([128, 2], F32, tag="mv", bufs=8, name="mv")
                    nc.vector.bn_aggr(out=mv, in_=stats)
                    rstd = work.tile([128, 1], F32, tag="rstd", bufs=8, name="rstd")
                    nc.scalar.activation(
                        out=rstd, in_=mv[:, 1:2],
                        func=mybir.ActivationFunctionType.Sqrt,
                        bias=eps_sb, scale=1.0,
                    )
                    nc.vector.reciprocal(out=rstd, in_=rstd)
                    xc = work.tile([128, D], BF16, tag="xc", bufs=8, name="xc")
                    nc.vector.tensor_scalar(
                        out=xc, in0=xt, scalar1=mv[:, 0:1], scalar2=rstd,
                        op0=mybir.AluOpType.subtract, op1=mybir.AluOpType.mult,
                    )
                    xcs.append(xc)
                for dc in range(DC):
                    tpg = tp_pool.tile([128, 512], BF16, name="tpg")
                    for i in range(4):
                        nc.tensor.transpose(
                            tpg[:, i * 128:(i + 1) * 128],
                            xcs[i][:, dc * 128:(dc + 1) * 128], identity,
                        )
                    nc.scalar.activation(
                        out=xnT[:, dc, g4 * 512:(g4 + 1) * 512], in_=tpg,
                        func=mybir.ActivationFunctionType.Identity,
                        bias=beta_sb[:, dc:dc + 1], scale=gamma_sb[:, dc:dc + 1],
                    )
                # projections for this 512-token range (PE-heavy)
                t4 = g4
                for p in range(4):  # K for head pair p
                    pk = pj_pool.tile([128, 512], F32, tag="proj", name="pk")
                    for dc in range(DC):
                        nc.tensor.matmul(
                            pk,
                            lhsT=wq_sb[:, dc, D + p * 128:D + (p + 1) * 128],
                            rhs=xnT[:, dc, t4 * 512:(t4 + 1) * 512],
                            start=(dc == 0), stop=(dc == DC - 1),
                        )
                    nc.scalar.copy(kt_all[:, p, t4 * 512:(t4 + 1) * 512], pk)
                if t4 < NQ // 512:
                    for p in range(4):  # Q for head pair p
                        pq = pj_pool.tile([128, 512], F32, tag="proj", name="pq")
                        for dc in range(DC):
                            nc.tensor.matmul(
                                pq,
                                lhsT=wq_sb[:, dc, p * 128:(p + 1) * 128],
                                rhs=xnT[:, dc, t4 * 512:(t4 + 1) * 512],
                                start=(dc == 0), stop=(dc == DC - 1),
                            )
                        nc.scalar.copy(qt_all[:, p, t4 * 512:(t4 + 1) * 512], pq)
                for tt in range(4 * g4, 4 * g4 + 4):  # V for all 8 heads
                    pv = pj_pool.tile([128, 512], F32, tag="proj", name="pv")
                    for dc in range(DC):
                        nc.tensor.matmul(
                            pv,
                            lhsT=xnT[:, dc, tt * 128:(tt + 1) * 128],
                            rhs=wq_sb[:, dc, 2 * D:3 * D],
                            start=(dc == 0), stop=(dc == DC - 1),
                        )
                    nc.vector.tensor_copy(
                        v_all[:, tt, :].rearrange("p (h e) -> p h e", e=DH + 1)[:, :, 0:DH],
                        pv.rearrange("p (h d) -> p h d", d=DH),
                    )

        # ---- Phase C: attention ----
        with (
            tc.tile_pool(name="st_psum", bufs=2, space="PSUM") as st_pool,
            tc.tile_pool(name="o_psum", bufs=2, space="PSUM") as o_pool,
        ):
            for qt in range(QT):
                for p in range(4):
                    oacc = [
                        o_pool.tile([DH + 1, 512], F32, tag=f"o{i}", name=f"o{i}")
                        for i in range(2)
                    ]
                    prev_at = None
                    for kc in range(KC + 1):
                        if kc < KC:
                            st = st_pool.tile([128, 1024], F32, name="st")
                            for half in range(2):
                                nc.tensor.matmul(
                                    st[:, half * 512:(half + 1) * 512],
                                    lhsT=kt_all[64 * half:64 * half + 64, p,
                                                kc * 128:(kc + 1) * 128],
                                    rhs=qt_all[64 * half:64 * half + 64, p,
                                               qt * 512:(qt + 1) * 512],
                                    start=True, stop=True,
                                )
                        if prev_at is not None:
                            pkc = kc - 1
                            for half in range(2):
                                h = 2 * p + half
                                nc.tensor.matmul(
                                    oacc[half],
                                    lhsT=v_all[:, pkc, h * (DH + 1):(h + 1) * (DH + 1)],
                                    rhs=prev_at[:, half * 512:(half + 1) * 512],
                                    start=(pkc == 0), stop=(pkc == KC - 1),
                                )
                        if kc < KC:
                            at = work.tile([128, 1024], BF16, tag="at", bufs=4, name="at")
                            nc.scalar.activation(
                                out=at, in_=st,
                                func=mybir.ActivationFunctionType.Exp, scale=SCALE,
                            )
                            prev_at = at
                    for half in range(2):
                        o_acc = oacc[half]
                        s_sb = work.tile([1, 512], F32, tag="s_sb", bufs=3, name="s_sb")
                        nc.vector.tensor_copy(s_sb, o_acc[DH:DH + 1, :])
                        r_sb = work.tile([1, 512], F32, tag="r_sb", bufs=3, name="r_sb")
                        nc.vector.reciprocal_approx_fast(out=r_sb, in_=s_sb)
                        cb_sb = work.tile([DH, 512], F32, tag="cb", bufs=3, name="cb")
                        nc.gpsimd.partition_broadcast(cb_sb, r_sb)
                        nc.vector.tensor_tensor(
                            ot_all[64 * half:64 * half + 64, p,
                                   qt * 512:(qt + 1) * 512],
                            o_acc[0:DH, :], cb_sb, mybir.AluOpType.mult,
                        )
        # ---- Phase D: output projection ----
        with tc.tile_pool(name="y_psum", bufs=2, space="PSUM") as y_pool:
            for qt in range(QT):
                for dm in range(DC):
                    yp = y_pool.tile([128, 512], F32, name="yp")
                    for p in range(4):
                        nc.tensor.matmul(
                            yp,
                            lhsT=wo_sb[:, p, dm * 128:(dm + 1) * 128],
                            rhs=ot_all[:, p, qt * 512:(qt + 1) * 512],
                            start=(p == 0), stop=(p == 3),
                        )
                    y_sb = work.tile([128, 512], F32, tag="y_sb", bufs=3, name="y_sb")
                    nc.vector.tensor_scalar_add(y_sb, yp, bias_sb[:, dm:dm + 1])
                    nc.sync.dma_start(
                        y_ap[dm * 128:(dm + 1) * 128, qt * 512:(qt + 1) * 512], y_sb
                    )


_CACHED_NC = None


def _get_program():
    global _CACHED_NC
    if _CACHED_NC is None:
        _CACHED_NC = build_program()
    return _CACHED_NC


def make_in_maps(x, ln_gamma, ln_beta, W_qkv, W_out, b_out):
    x = np.asarray(x, dtype=np.float32)
    wqkv_bf = np.asarray(W_qkv, dtype=np.float32).astype(ml_dtypes.bfloat16)
    wout_bf = np.asarray(W_out, dtype=np.float32).astype(ml_dtypes.bfloat16)
    bout = np.asarray(b_out, dtype=np.float32)
    gamma = np.asarray(ln_gamma, dtype=np.float32)
    beta = np.asarray(ln_beta, dtype=np.float32)
    in_maps = []
    for c in range(NCORES):
        b, qh = c // 2, c % 2
        xb = np.roll(x[b], -NQ * qh, axis=0)  # query rows first
        in_maps.append({
            "x": np.ascontiguousarray(xb),
            "w_qkv": wqkv_bf,
            "w_out": wout_bf,
            "b_out": bout,
            "gamma": gamma,
            "beta": beta,
        })
    return in_maps


def kernel(x, ln_gamma, ln_beta, W_qkv, W_out, b_out):
    nc = _get_program()
    in_maps = make_in_maps(x, ln_gamma, ln_beta, W_qkv, W_out, b_out)
    res = run_bass_kernel_spmd(nc, in_maps, core_ids=list(range(NCORES)))

    y = np.empty((B, N, D), dtype=np.float32)
    for c in range(NCORES):
        b, qh = c // 2, c % 2
        y[b, NQ * qh:NQ * (qh + 1), :] = res.results[c]["y_t"].T
    return y
